# revision 73
# baseline (speedup 1.0000x reference)
"""Trainium2 Bass kernel for causal MHSA (B=2, S=2048, D=1024, H=16, HD=64).

Sharding: 8 cores = 2 (batch) x 4 (head-groups of 4 heads). Each core: QKV
projections for its 4 heads, causal flash attention, partial out-projection
(its 256 columns of o_w). Host sums the 4 bf16 partials per batch in f32.

Design (optimized against the TimelineSim cost model; 158us -> 116.6us):
  - AV matmul uses P^T chunks as the STATIONARY operand and V-augmented
    (64 cols + ones col) as the MOVING operand -> natural-layout O [q, hd+1]
    at 65 streamed cols per (head, q-subtile, kt) instead of 512, and the
    softmax denominator lands as a per-partition column (cheap reciprocal +
    per-partition tensor_scalar normalize; no partition_broadcast).
  - Fully-masked (q-subtile, kt) pairs are skipped entirely (no AV matmul,
    no zero-fill memsets); only the diagonal subtile needs a masked copy of
    P^T ([128,128] tensor_mul against one lower-tri tile).
  - O_n (normalized, bf16) is transposed to OT via DMA-engine transpose
    (14ns/xbar-tile, no PE/PSUM involvement) for the out-projection.
  - Q/K tiles in bf16 (1 PE cycle/row at any moving size; f32r pays 4x at
    N=128 diagonal chunks).
  - Projections and out-projection are paced as "filler" PE work between
    score chunks across the whole kernel so PE never starves while ACT
    (exp) digests; scores stay ~2 chunks ahead of exp via a 2-deep PSUM
    ring, pT ring decouples AV.
  - Input DMAs merged (one per weight matrix, x split per-d for the first
    s-block to chase arrivals) to cut HWDGE serialization.
"""

import sys

if "/opt/trn_rl_repo" not in sys.path:
    sys.path.insert(0, "/opt/trn_rl_repo")

from collections import deque
from contextlib import ExitStack

import ml_dtypes
import numpy as np

import concourse.mybir as mybir
import concourse.tile as tile
from concourse import bacc
from concourse.bass_utils import run_bass_kernel_spmd

F32 = mybir.dt.float32
BF16 = mybir.dt.bfloat16

B, S, D, H = 2, 2048, 1024, 16
HD = D // H  # 64
N_CORES = 8
HPC = 4  # heads per core
DQ = HPC * HD  # 256 local qkv dims per core
SB = 512  # q block
KT = 128  # k tile
NQB = S // SB  # 4
NST = S // KT  # 16 s-tiles
VC = HD + 1  # V cols per head incl ones column

# Emission-schedule knobs (tuned against TimelineSim). `fillers` pins each
# projection / out-projection unit to a global score-chunk slot; placement
# must respect emission-order data dependencies (see build_nc).
CONFIG = {
    "lag": 4,
    "lag_tail": 4,
    "tail_from": 73,
    "stretch_order": [(0, 0), (0, 1), (1, 0), (1, 1), (2, 0), (2, 1), (3, 0), (3, 1)],
    "fillers": [
        (1, "v0"), (2, "v1"), (2, "v2"), (3, "v3"),
        (6, "q10"), (7, "k10"), (8, "v4"), (10, "v5"), (12, "q11"), (12, "v6"),
        (14, "k11"), (14, "v7"), (18, "q20"), (20, "k20"), (25, "v8"),
        (28, "v9"), (30, "q21"), (31, "v10"), (32, "k21"), (33, "v11"),
        (38, "o0"), (40, "o1"), (42, "o2"), (44, "o3"), (46, "q30"), (47, "k30"),
        (50, "v12"), (53, "v13"), (56, "q31"), (56, "o4"),
        (57, "v14"), (58, "k31"), (60, "v15"), (60, "o5"),
        (66, "o6"), (69, "o7"), (72, "o8"), (74, "o9"), (76, "o10"), (78, "o11"),
    ],
}


def build_nc():
    nc = bacc.Bacc("TRN2", target_bir_lowering=False, debug=False, num_devices=N_CORES)
    xT_h = nc.dram_tensor("xT", [D, S], BF16, kind="ExternalInput")
    wqT_h = nc.dram_tensor("wqT", [D, DQ], BF16, kind="ExternalInput")
    wkT_h = nc.dram_tensor("wkT", [D, DQ], BF16, kind="ExternalInput")
    wvT_h = nc.dram_tensor("wvT", [D, DQ], BF16, kind="ExternalInput")
    woT_h = nc.dram_tensor("woT", [2, 128, D], BF16, kind="ExternalInput")
    tri_h = nc.dram_tensor("tri", [KT, KT], BF16, kind="ExternalInput")
    eye_h = nc.dram_tensor("eye", [KT, KT], BF16, kind="ExternalInput")
    y_h = nc.dram_tensor("y", [S, D], BF16, kind="ExternalOutput")

    with tile.TileContext(nc) as tc, ExitStack() as ctx:
        persist = ctx.enter_context(tc.tile_pool(name="persist", bufs=1))
        QT = [persist.tile([128, S], BF16, tag=f"QT{t}", name=f"QT{t}") for t in range(2)]
        KTt = [persist.tile([128, S], BF16, tag=f"KT{t}", name=f"KT{t}") for t in range(2)]
        V = [persist.tile([128, HPC * VC], BF16, tag=f"V{i}", name=f"V{i}") for i in range(NST)]
        OT = [persist.tile([128, S], BF16, tag=f"OT{t}", name=f"OT{t}") for t in range(2)]
        wo_sb = persist.tile([128, 2 * D], BF16, tag="wo", name="wo_sb")
        tri = persist.tile([KT, KT], BF16, tag="tri", name="tri")
        eye = persist.tile([KT, KT], BF16, tag="eye", name="eye")
        wsrc = persist.tile([KT, KT], BF16, tag="wsrc", name="wsrc")
        x_sb = persist.tile([128, 8 * S], BF16, tag="x", name="x_sb")
        wq_sb = persist.tile([128, 8 * DQ], BF16, tag="wq", name="wq_sb")
        wk_sb = persist.tile([128, 8 * DQ], BF16, tag="wk", name="wk_sb")
        wv_sb = persist.tile([128, 8 * DQ], BF16, tag="wv", name="wv_sb")

        x3 = x_sb.rearrange("p (a s) -> p a s", a=8)

        pTp = ctx.enter_context(tc.tile_pool(name="pTp", bufs=8))
        pTmp = ctx.enter_context(tc.tile_pool(name="pTmp", bufs=4))
        onp = ctx.enter_context(tc.tile_pool(name="onp", bufs=8))
        rcp = ctx.enter_context(tc.tile_pool(name="rcp", bufs=8))
        ysb = ctx.enter_context(tc.tile_pool(name="ysb", bufs=6))
        psS = ctx.enter_context(tc.tile_pool(name="psS", bufs=2, space="PSUM"))
        psO = ctx.enter_context(tc.tile_pool(name="psO", bufs=1, space="PSUM"))
        psW = ctx.enter_context(tc.tile_pool(name="psW", bufs=2, space="PSUM"))

        # ---------------- input DMAs ----------------
        nc.sync.dma_start(tri[:], tri_h[:, :])
        nc.sync.dma_start(
            wq_sb.rearrange("p (a c) -> p a c", a=8),
            wqT_h.rearrange("(a p) c -> p a c", p=128),
        )
        nc.sync.dma_start(
            wk_sb.rearrange("p (a c) -> p a c", a=8),
            wkT_h.rearrange("(a p) c -> p a c", p=128),
        )
        # x s-block 0 per d-tile (chase arrivals for Q0/K0 pipeline)
        for d in range(8):
            nc.sync.dma_start(x3[:, d, 0:SB], xT_h[d * 128 : (d + 1) * 128, 0:SB])
        nc.sync.dma_start(
            wv_sb.rearrange("p (a c) -> p a c", a=8),
            wvT_h.rearrange("(a p) c -> p a c", p=128),
        )
        for blk in range(1, 4):
            ssl = slice(blk * SB, (blk + 1) * SB)
            nc.sync.dma_start(
                x3[:, :, ssl],
                xT_h.rearrange("(a p) s -> p a s", p=128)[:, :, ssl],
            )
        nc.sync.dma_start(eye[:], eye_h[:, :])
        nc.sync.dma_start(
            wo_sb.rearrange("p (v d) -> p v d", v=2),
            woT_h.rearrange("v p d -> p v d"),
        )

        # ---------------- emission helpers ----------------
        def w_tile(w_sb, d, t):
            return w_sb[:, d * DQ + t * 128 : d * DQ + (t + 1) * 128]

        def proj_qk(w_sb, dst, blk, t, scale):
            ps = psW.tile([128, SB], F32, tag="pw", name="ps_qk")
            ssl = slice(blk * SB, (blk + 1) * SB)
            for d in range(8):
                nc.tensor.matmul(
                    ps[:],
                    w_tile(w_sb, d, t),
                    x3[:, d, ssl],
                    start=(d == 0),
                    stop=(d == 7),
                    skip_group_check=True,
                )
            if scale is not None:
                nc.vector.tensor_scalar_mul(dst[t][:, ssl], ps[:], scale)
            else:
                nc.vector.tensor_copy(dst[t][:, ssl], ps[:])

        def proj_v(st):
            pv = psW.tile([128, DQ], F32, tag="pw", name="pv", padded_shape=[128, SB])
            tsl = slice(st * KT, (st + 1) * KT)
            for d in range(8):
                nc.tensor.matmul(
                    pv[:],
                    x3[:, d, tsl],
                    wv_sb[:, d * DQ : (d + 1) * DQ],
                    start=(d == 0),
                    stop=(d == 7),
                    skip_group_check=True,
                )
            v5 = V[st].rearrange("p (h c) -> p h c", c=VC)
            nc.vector.tensor_copy(v5[:, :, 0:HD], pv.rearrange("p (h c) -> p h c", c=HD))
            nc.gpsimd.memset(v5[:, :, HD:VC], 1.0)

        def oproj(st):
            ssl = slice(st * KT, (st + 1) * KT)
            yj = [psW.tile([128, SB], F32, tag="pw", name=f"yj{j}") for j in range(2)]
            for j in range(2):
                for v in range(2):
                    nc.tensor.matmul(
                        yj[j][:],
                        OT[v][:, ssl],
                        wo_sb[:, v * D + j * SB : v * D + (j + 1) * SB],
                        start=(v == 0),
                        stop=(v == 1),
                        skip_group_check=True,
                    )
            y_t = ysb.tile([128, D], BF16, tag="ysb", name="y_t")
            nc.vector.tensor_copy(y_t[:, 0:SB], yj[0][:])
            nc.vector.tensor_copy(y_t[:, SB:D], yj[1][:])
            nc.sync.dma_start(y_h[ssl, :], y_t[:])

        def oproj_final(st):
            # Tail variant: runs in the drain, after all score chunks — the
            # score PSUM ring is free, so each final's two y accumulators
            # share one 2-bank "sc" slot (finals double-buffer instead of
            # chaining through the pw ring), and ACT (idle post-exp) takes
            # one of the staging copies.
            ssl = slice(st * KT, (st + 1) * KT)
            ypf = psS.tile([128, 2 * SB], F32, tag="sc", name="ypf")
            for j in range(2):
                for v in range(2):
                    nc.tensor.matmul(
                        ypf[:, j * SB : (j + 1) * SB],
                        OT[v][:, ssl],
                        wo_sb[:, v * D + j * SB : v * D + (j + 1) * SB],
                        start=(v == 0),
                        stop=(v == 1),
                        skip_group_check=True,
                    )
            y_t = ysb.tile([128, D], BF16, tag="ysb", name="y_t")
            nc.scalar.copy(y_t[:, 0:SB], ypf[:, 0:SB])
            nc.vector.tensor_copy(y_t[:, SB:D], ypf[:, SB:])
            nc.sync.dma_start(y_h[ssl, :], y_t[:])

        def emit_chunk(qb, pair, kt):
            """Scores + exp (+ diag mask) for chunk (qb, pair, kt).
            Returns (pT, pTm) bf16 tiles; pT cols [hh*SB+off, (hh+1)*SB)."""
            m = kt - 4 * qb
            off = KT * m if m >= 1 else 0
            ksl = slice(kt * KT, (kt + 1) * KT)
            sps = psS.tile([128, 2 * SB], F32, tag="sc", name="sc")
            for hh in range(2):
                hsl = slice(hh * HD, (hh + 1) * HD)
                nc.tensor.matmul(
                    sps[:, hh * SB + off : (hh + 1) * SB],
                    KTt[pair][hsl, ksl],
                    QT[pair][hsl, qb * SB + off : (qb + 1) * SB],
                    start=True,
                    stop=True,
                    skip_group_check=True,
                )
            pT = pTp.tile([128, 2 * SB], BF16, tag="pT", name="pT")
            if off:
                s3 = sps.rearrange("p (r f) -> p r f", r=2)[:, :, off:]
                p3 = pT.rearrange("p (r f) -> p r f", r=2)[:, :, off:]
                nc.scalar.activation(p3, s3, mybir.ActivationFunctionType.Exp)
            else:
                nc.scalar.activation(pT[:], sps[:], mybir.ActivationFunctionType.Exp)
            pTm = None
            if 0 <= m:
                # chain-critical diag chunks of the last stretch mask on DVE
                # (shorter op, off the Pool q7-launch path); the rest on Pool
                eng = nc.vector if (qb == 3 and pair == 1) else nc.gpsimd
                pTm = pTmp.tile([128, 2 * KT], BF16, tag="pTm", name="pTm")
                for hh in range(2):
                    eng.tensor_mul(
                        pTm[:, hh * KT : (hh + 1) * KT],
                        pT[:, hh * SB + m * KT : hh * SB + (m + 1) * KT],
                        tri[:],
                    )
            return pT, pTm

        def emit_av(qb, pair, kt, oa, pT, pTm):
            """AV for chunk kt into oa; on the diagonal qsub, also emits
            normalization + transpose for that finished q-subtile."""
            m = kt - 4 * qb
            on_t = None
            if 0 <= m:
                on_t = onp.tile([128, KT], BF16, tag="on", name="on")
            for hh in range(2):
                h = 2 * pair + hh
                for qsub in range(4):
                    if qsub < m:
                        continue
                    if qsub == m:
                        lhs = pTm[:, hh * KT : (hh + 1) * KT]
                    else:
                        lhs = pT[:, hh * SB + qsub * KT : hh * SB + (qsub + 1) * KT]
                    # One accumulation group per PSUM bank (2KB zero region):
                    # start only on the first matmul into the bank, stop on
                    # the last. Per-region zero-init happens on first touch
                    # via the pending-zero mechanics.
                    nc.tensor.matmul(
                        oa[hh][:, qsub * VC : (qsub + 1) * VC],
                        lhs,
                        V[kt][:, h * VC : (h + 1) * VC],
                        start=(kt == 0 and qsub == 0),
                        stop=(kt == 4 * qb + 3 and qsub == 3),
                        skip_group_check=True,
                    )
            if 0 <= m:
                # q-subtile `m` just received its last contribution: normalize
                qsl_g = slice((qb * 4 + m) * KT, (qb * 4 + m + 1) * KT)
                for hh in range(2):
                    rc = rcp.tile([128, 1], F32, tag="rc", name="rc")
                    nc.vector.reciprocal(
                        rc[:], oa[hh][:, m * VC + HD : m * VC + VC]
                    )
                    nc.vector.tensor_scalar_mul(
                        on_t[:, hh * HD : (hh + 1) * HD],
                        oa[hh][:, m * VC : m * VC + HD],
                        rc[:],
                    )
                if qb == 3 and pair == 1:
                    # Final stretch: DMA-transpose latency (~2.4us) would
                    # gate the final out-projections; PE has slack here, so
                    # transpose on the PE via identity permutation. The
                    # out-projection is returned for deferred emission so
                    # the next chunk's scores aren't stuck behind it.
                    tp_ps = psW.tile([128, KT], BF16, tag="pw", name="tp_ps")
                    nc.tensor.matmul(
                        tp_ps[:], on_t[:], eye[:],
                        is_transpose=True, start=True, stop=True,
                        skip_group_check=True,
                    )
                    nc.vector.tensor_copy(OT[pair][:, qsl_g], tp_ps[:])
                    return lambda: oproj_final(12 + m)
                else:
                    nc.sync.dma_start_transpose(OT[pair][:, qsl_g], on_t[:])
            return None

        # ---------------- prologue: Q0/K0 (both t) chase x DMA arrivals ------
        # PE p-state ramps to full speed only after ~3us of CONTINUOUS busy;
        # any idle gap resets it. Keep PE spinning on throwaway matmuls while
        # input DMAs stream so the real work runs at 2.4GHz from the start.
        # Q0/K0 for BOTH pairs accumulate into the (idle) score PSUM slots:
        # 4 passes per d-tile arrival keeps PE saturated through the chase.
        # warm source comes from a memset (not a DMA) so PE can start
        # spinning at ~0.3us instead of waiting ~3us for the first DMA
        nc.vector.memset(wsrc[:], 1.0)

        def warm(n):
            for _ in range(n):
                wps = psW.tile([128, KT], F32, tag="pw", name="wps")
                nc.tensor.matmul(
                    wps[:], wsrc[:], wsrc[:], start=True, stop=True,
                    skip_group_check=True,
                )

        warm(44)
        psq = psS.tile([128, 2 * SB], F32, tag="sc", name="psq")
        psk = psS.tile([128, 2 * SB], F32, tag="sc", name="psk")
        for d in range(8):
            for t in range(2):
                nc.tensor.matmul(
                    psq[:, t * SB : (t + 1) * SB], w_tile(wq_sb, d, t),
                    x3[:, d, 0:SB],
                    start=(d == 0), stop=(d == 7), skip_group_check=True,
                )
                nc.tensor.matmul(
                    psk[:, t * SB : (t + 1) * SB], w_tile(wk_sb, d, t),
                    x3[:, d, 0:SB],
                    start=(d == 0), stop=(d == 7), skip_group_check=True,
                )
        # ACT is idle until the first exp — it takes the K copies so the
        # first score chunk isn't serialized behind a DVE copy queue.
        nc.scalar.copy(KTt[0][:, 0:SB], psk[:, 0:SB])
        nc.vector.tensor_scalar_mul(QT[0][:, 0:SB], psq[:, 0:SB], 0.125)
        nc.scalar.copy(KTt[1][:, 0:SB], psk[:, SB:])
        nc.vector.tensor_scalar_mul(QT[1][:, 0:SB], psq[:, SB:], 0.125)

        # ---------------- global schedule ----------------
        # Scores/exp for ALL chunks flow as one global stream (decoupled
        # from AV consumption via the SBUF pT ring) so ACT never starves at
        # (qb, pair) stretch boundaries. AV lags LAG chunks behind and
        # switches oa PSUM groups as its pointer crosses stretch borders.
        # Fillers are pinned to global chunk-slots, respecting emission-
        # order dependencies (V(st) before its first AV, Q/K(blk, t) before
        # that stretch's first scores, oproj after its OT transposes).
        def pv(st):
            return lambda: proj_v(st)

        def pq(blk, t):
            return lambda: proj_qk(wq_sb, QT, blk, t, 0.125)

        def pk(blk, t):
            return lambda: proj_qk(wk_sb, KTt, blk, t, None)

        def po(st):
            return lambda: oproj(st)

        unit_map = {}
        for st in range(16):
            unit_map[f"v{st}"] = pv(st)
            if st < 12:
                unit_map[f"o{st}"] = po(st)
        for blk in range(4):
            for t in range(2):
                unit_map[f"q{blk}{t}"] = pq(blk, t)
                unit_map[f"k{blk}{t}"] = pk(blk, t)
        fillers = [(s, unit_map[u]) for s, u in CONFIG["fillers"]]
        fillers.sort(key=lambda x: x[0])

        chunks = [
            (qb, pair, kt)
            for qb, pair in CONFIG["stretch_order"]
            for kt in range(4 * (qb + 1))
        ]
        LAG = CONFIG["lag"]
        filled = 0
        cur_group = None
        oa = None
        pending = deque()
        post_q = deque()

        def fill_to(s):
            nonlocal filled
            while filled < len(fillers) and fillers[filled][0] <= s:
                fillers[filled][1]()
                filled += 1

        def process_av():
            nonlocal cur_group, oa
            (qb_, pair_, kt_), pT_, pTm_ = pending.popleft()
            if cur_group != (qb_, pair_):
                cur_group = (qb_, pair_)
                oa = [
                    psO.tile(
                        [128, 4 * VC], F32, tag=f"oa{hh}", name=f"oa{hh}",
                        padded_shape=[128, SB],
                    )
                    for hh in range(2)
                ]
            post = emit_av(qb_, pair_, kt_, oa, pT_, pTm_)
            if post is not None:
                post_q.append(post)

        lag_tail = CONFIG.get("lag_tail", LAG)
        tail_from = CONFIG.get("tail_from", len(chunks) + 1)
        si = 0
        for si, (qb, pair, kt) in enumerate(chunks, start=1):
            pT, pTm = emit_chunk(qb, pair, kt)
            pending.append(((qb, pair, kt), pT, pTm))
            fill_to(si)
            while post_q:
                post_q.popleft()()
            while len(pending) > (lag_tail if si >= tail_from else LAG):
                process_av()
        while pending:
            process_av()
            si += 1
            fill_to(si)
            while post_q:
                post_q.popleft()()
        fill_to(10**9)

        # (st 12-15 out-projections are emitted inline at qb3/pair1 diag AVs)

    nc.compile()
    return nc


_NC = None


def _get_nc():
    global _NC
    if _NC is None:
        _NC = build_nc()
    return _NC


def make_in_maps(x, q_w, k_w, v_w, o_w):
    kk = np.arange(KT)[:, None]
    qq = np.arange(KT)[None, :]
    tri = (kk <= qq).astype(ml_dtypes.bfloat16)
    in_maps = []
    for c in range(N_CORES):
        b, g = c // 4, c % 4
        rows = slice(g * DQ, (g + 1) * DQ)
        woT = np.ascontiguousarray(o_w[:, g * DQ : (g + 1) * DQ].T).reshape(2, 128, D)
        eye = np.eye(KT, dtype=ml_dtypes.bfloat16)
        in_maps.append(
            {
                "xT": np.ascontiguousarray(x[b].T).astype(ml_dtypes.bfloat16),
                "wqT": np.ascontiguousarray(q_w[rows, :].T).astype(ml_dtypes.bfloat16),
                "wkT": np.ascontiguousarray(k_w[rows, :].T).astype(ml_dtypes.bfloat16),
                "wvT": np.ascontiguousarray(v_w[rows, :].T).astype(ml_dtypes.bfloat16),
                "woT": woT.astype(ml_dtypes.bfloat16),
                "tri": tri,
                "eye": eye,
            }
        )
    return in_maps


def run(x, q_w, k_w, v_w, o_w, trace=False, **spmd_kwargs):
    nc = _get_nc()
    in_maps = make_in_maps(
        np.asarray(x, dtype=np.float32),
        np.asarray(q_w, dtype=np.float32),
        np.asarray(k_w, dtype=np.float32),
        np.asarray(v_w, dtype=np.float32),
        np.asarray(o_w, dtype=np.float32),
    )
    res = run_bass_kernel_spmd(
        nc, in_maps, core_ids=list(range(N_CORES)), trace=trace, **spmd_kwargs
    )
    parts = [np.asarray(r["y"], dtype=np.float32) for r in res.results]
    out = np.empty((B, S, D), dtype=np.float32)
    for b in range(B):
        out[b] = parts[b * 4] + parts[b * 4 + 1] + parts[b * 4 + 2] + parts[b * 4 + 3]
    return out, res


def kernel(x, q_w, k_w, v_w, o_w):
    out, _ = run(x, q_w, k_w, v_w, o_w, trace=False)
    return out


# revision 74
# speedup vs baseline: 1.0143x; 1.0143x over previous
"""Trainium2 Bass kernel for causal MHSA (B=2, S=2048, D=1024, H=16, HD=64).

Sharding: 8 cores = 2 (batch) x 4 (head-groups of 4 heads). Each core: QKV
projections for its 4 heads, causal flash attention, partial out-projection
(its 256 columns of o_w). Host sums the 4 bf16 partials per batch in f32.

Design (optimized against the TimelineSim cost model; 158us -> 116.6us):
  - AV matmul uses P^T chunks as the STATIONARY operand and V-augmented
    (64 cols + ones col) as the MOVING operand -> natural-layout O [q, hd+1]
    at 65 streamed cols per (head, q-subtile, kt) instead of 512, and the
    softmax denominator lands as a per-partition column (cheap reciprocal +
    per-partition tensor_scalar normalize; no partition_broadcast).
  - Fully-masked (q-subtile, kt) pairs are skipped entirely (no AV matmul,
    no zero-fill memsets); only the diagonal subtile needs a masked copy of
    P^T ([128,128] tensor_mul against one lower-tri tile).
  - O_n (normalized, bf16) is transposed to OT via DMA-engine transpose
    (14ns/xbar-tile, no PE/PSUM involvement) for the out-projection.
  - Q/K tiles in bf16 (1 PE cycle/row at any moving size; f32r pays 4x at
    N=128 diagonal chunks).
  - Projections and out-projection are paced as "filler" PE work between
    score chunks across the whole kernel so PE never starves while ACT
    (exp) digests; scores stay ~2 chunks ahead of exp via a 2-deep PSUM
    ring, pT ring decouples AV.
  - Input DMAs merged (one per weight matrix, x split per-d for the first
    s-block to chase arrivals) to cut HWDGE serialization.
"""

import sys

if "/opt/trn_rl_repo" not in sys.path:
    sys.path.insert(0, "/opt/trn_rl_repo")

from collections import deque
from contextlib import ExitStack

import ml_dtypes
import numpy as np

import concourse.mybir as mybir
import concourse.tile as tile
from concourse import bacc
from concourse.bass_utils import run_bass_kernel_spmd

F32 = mybir.dt.float32
BF16 = mybir.dt.bfloat16

B, S, D, H = 2, 2048, 1024, 16
HD = D // H  # 64
N_CORES = 8
HPC = 4  # heads per core
DQ = HPC * HD  # 256 local qkv dims per core
SB = 512  # q block
KT = 128  # k tile
NQB = S // SB  # 4
NST = S // KT  # 16 s-tiles
VC = HD + 1  # V cols per head incl ones column

# Emission-schedule knobs (tuned against TimelineSim). `fillers` pins each
# projection / out-projection unit to a global score-chunk slot; placement
# must respect emission-order data dependencies (see build_nc).
CONFIG = {
    "lag": 5,
    "lag_tail": 5,
    "tail_from": 73,
    "stretch_order": [(0, 0), (0, 1), (1, 0), (1, 1), (2, 0), (2, 1), (3, 0), (3, 1)],
    "fillers": [
        (1, "v0"), (2, "v1"), (2, "v2"), (3, "v3"),
        (6, "q10"), (7, "k10"), (8, "v4"), (10, "v5"), (12, "q11"), (12, "v6"),
        (14, "k11"), (14, "v7"), (18, "q20"), (20, "k20"), (25, "v8"),
        (28, "v9"), (30, "q21"), (31, "v10"), (32, "k21"), (33, "v11"),
        (38, "o0"), (40, "o1"), (42, "o2"), (44, "o3"), (46, "q30"), (47, "k30"),
        (50, "v12"), (53, "v13"), (56, "q31"), (56, "o4"),
        (57, "v14"), (58, "k31"), (60, "v15"), (62, "o5"),
        (64, "o6"), (67, "o7"), (70, "o8"), (73, "o9"), (76, "o10"), (79, "o11"),
    ],
}


def build_nc():
    nc = bacc.Bacc("TRN2", target_bir_lowering=False, debug=False, num_devices=N_CORES)
    xT_h = nc.dram_tensor("xT", [D, S], BF16, kind="ExternalInput")
    wqT_h = nc.dram_tensor("wqT", [D, DQ], BF16, kind="ExternalInput")
    wkT_h = nc.dram_tensor("wkT", [D, DQ], BF16, kind="ExternalInput")
    wvT_h = nc.dram_tensor("wvT", [D, DQ], BF16, kind="ExternalInput")
    woT_h = nc.dram_tensor("woT", [2, 128, D], BF16, kind="ExternalInput")
    tri_h = nc.dram_tensor("tri", [KT, KT], BF16, kind="ExternalInput")
    eye_h = nc.dram_tensor("eye", [KT, KT], BF16, kind="ExternalInput")
    y_h = nc.dram_tensor("y", [S, D], BF16, kind="ExternalOutput")

    with tile.TileContext(nc) as tc, ExitStack() as ctx:
        persist = ctx.enter_context(tc.tile_pool(name="persist", bufs=1))
        QT = [persist.tile([128, S], BF16, tag=f"QT{t}", name=f"QT{t}") for t in range(2)]
        KTt = [persist.tile([128, S], BF16, tag=f"KT{t}", name=f"KT{t}") for t in range(2)]
        V = [persist.tile([128, HPC * VC], BF16, tag=f"V{i}", name=f"V{i}") for i in range(NST)]
        OT = [persist.tile([128, S], BF16, tag=f"OT{t}", name=f"OT{t}") for t in range(2)]
        wo_sb = persist.tile([128, 2 * D], BF16, tag="wo", name="wo_sb")
        tri = persist.tile([KT, KT], BF16, tag="tri", name="tri")
        eye = persist.tile([KT, KT], BF16, tag="eye", name="eye")
        wsrc = persist.tile([KT, KT], BF16, tag="wsrc", name="wsrc")
        x_sb = persist.tile([128, 8 * S], BF16, tag="x", name="x_sb")
        wq_sb = persist.tile([128, 8 * DQ], BF16, tag="wq", name="wq_sb")
        wk_sb = persist.tile([128, 8 * DQ], BF16, tag="wk", name="wk_sb")
        wv_sb = persist.tile([128, 8 * DQ], BF16, tag="wv", name="wv_sb")

        x3 = x_sb.rearrange("p (a s) -> p a s", a=8)

        pTp = ctx.enter_context(tc.tile_pool(name="pTp", bufs=8))
        pTmp = ctx.enter_context(tc.tile_pool(name="pTmp", bufs=4))
        onp = ctx.enter_context(tc.tile_pool(name="onp", bufs=8))
        rcp = ctx.enter_context(tc.tile_pool(name="rcp", bufs=8))
        ysb = ctx.enter_context(tc.tile_pool(name="ysb", bufs=6))
        psS = ctx.enter_context(tc.tile_pool(name="psS", bufs=2, space="PSUM"))
        psO = ctx.enter_context(tc.tile_pool(name="psO", bufs=1, space="PSUM"))
        psW = ctx.enter_context(tc.tile_pool(name="psW", bufs=2, space="PSUM"))

        # ---------------- input DMAs ----------------
        nc.sync.dma_start(tri[:], tri_h[:, :])
        nc.sync.dma_start(
            wq_sb.rearrange("p (a c) -> p a c", a=8),
            wqT_h.rearrange("(a p) c -> p a c", p=128),
        )
        nc.sync.dma_start(
            wk_sb.rearrange("p (a c) -> p a c", a=8),
            wkT_h.rearrange("(a p) c -> p a c", p=128),
        )
        # x s-block 0 per d-tile (chase arrivals for Q0/K0 pipeline)
        for d in range(8):
            nc.sync.dma_start(x3[:, d, 0:SB], xT_h[d * 128 : (d + 1) * 128, 0:SB])
        nc.sync.dma_start(
            wv_sb.rearrange("p (a c) -> p a c", a=8),
            wvT_h.rearrange("(a p) c -> p a c", p=128),
        )
        for blk in range(1, 4):
            ssl = slice(blk * SB, (blk + 1) * SB)
            nc.sync.dma_start(
                x3[:, :, ssl],
                xT_h.rearrange("(a p) s -> p a s", p=128)[:, :, ssl],
            )
        nc.sync.dma_start(eye[:], eye_h[:, :])
        nc.sync.dma_start(
            wo_sb.rearrange("p (v d) -> p v d", v=2),
            woT_h.rearrange("v p d -> p v d"),
        )

        # ---------------- emission helpers ----------------
        def w_tile(w_sb, d, t):
            return w_sb[:, d * DQ + t * 128 : d * DQ + (t + 1) * 128]

        def proj_qk(w_sb, dst, blk, t, scale):
            ps = psW.tile([128, SB], F32, tag="pw", name="ps_qk")
            ssl = slice(blk * SB, (blk + 1) * SB)
            for d in range(8):
                nc.tensor.matmul(
                    ps[:],
                    w_tile(w_sb, d, t),
                    x3[:, d, ssl],
                    start=(d == 0),
                    stop=(d == 7),
                    skip_group_check=True,
                )
            if scale is not None:
                nc.vector.tensor_scalar_mul(dst[t][:, ssl], ps[:], scale)
            else:
                nc.vector.tensor_copy(dst[t][:, ssl], ps[:])

        def proj_v(st):
            pv = psW.tile([128, DQ], F32, tag="pw", name="pv", padded_shape=[128, SB])
            tsl = slice(st * KT, (st + 1) * KT)
            for d in range(8):
                nc.tensor.matmul(
                    pv[:],
                    x3[:, d, tsl],
                    wv_sb[:, d * DQ : (d + 1) * DQ],
                    start=(d == 0),
                    stop=(d == 7),
                    skip_group_check=True,
                )
            v5 = V[st].rearrange("p (h c) -> p h c", c=VC)
            nc.vector.tensor_copy(v5[:, :, 0:HD], pv.rearrange("p (h c) -> p h c", c=HD))
            nc.gpsimd.memset(v5[:, :, HD:VC], 1.0)

        def oproj(st):
            ssl = slice(st * KT, (st + 1) * KT)
            yj = [psW.tile([128, SB], F32, tag="pw", name=f"yj{j}") for j in range(2)]
            for j in range(2):
                for v in range(2):
                    nc.tensor.matmul(
                        yj[j][:],
                        OT[v][:, ssl],
                        wo_sb[:, v * D + j * SB : v * D + (j + 1) * SB],
                        start=(v == 0),
                        stop=(v == 1),
                        skip_group_check=True,
                    )
            y_t = ysb.tile([128, D], BF16, tag="ysb", name="y_t")
            nc.vector.tensor_copy(y_t[:, 0:SB], yj[0][:])
            nc.vector.tensor_copy(y_t[:, SB:D], yj[1][:])
            nc.sync.dma_start(y_h[ssl, :], y_t[:])

        def oproj_final(st):
            # Tail variant: runs in the drain, after all score chunks — the
            # score PSUM ring is free, so each final's two y accumulators
            # share one 2-bank "sc" slot (finals double-buffer instead of
            # chaining through the pw ring), and ACT (idle post-exp) takes
            # one of the staging copies.
            ssl = slice(st * KT, (st + 1) * KT)
            ypf = psS.tile([128, 2 * SB], F32, tag="sc", name="ypf")
            for j in range(2):
                for v in range(2):
                    nc.tensor.matmul(
                        ypf[:, j * SB : (j + 1) * SB],
                        OT[v][:, ssl],
                        wo_sb[:, v * D + j * SB : v * D + (j + 1) * SB],
                        start=(v == 0),
                        stop=(v == 1),
                        skip_group_check=True,
                    )
            y_t = ysb.tile([128, D], BF16, tag="ysb", name="y_t")
            nc.scalar.copy(y_t[:, 0:SB], ypf[:, 0:SB])
            nc.vector.tensor_copy(y_t[:, SB:D], ypf[:, SB:])
            nc.sync.dma_start(y_h[ssl, :], y_t[:])

        def emit_chunk(qb, pair, kt):
            """Scores + exp (+ diag mask) for chunk (qb, pair, kt).
            Returns (pT, pTm) bf16 tiles; pT cols [hh*SB+off, (hh+1)*SB)."""
            m = kt - 4 * qb
            off = KT * m if m >= 1 else 0
            ksl = slice(kt * KT, (kt + 1) * KT)
            sps = psS.tile([128, 2 * SB], F32, tag="sc", name="sc")
            for hh in range(2):
                hsl = slice(hh * HD, (hh + 1) * HD)
                nc.tensor.matmul(
                    sps[:, hh * SB + off : (hh + 1) * SB],
                    KTt[pair][hsl, ksl],
                    QT[pair][hsl, qb * SB + off : (qb + 1) * SB],
                    start=True,
                    stop=True,
                    skip_group_check=True,
                )
            pT = pTp.tile([128, 2 * SB], BF16, tag="pT", name="pT")
            if off:
                s3 = sps.rearrange("p (r f) -> p r f", r=2)[:, :, off:]
                p3 = pT.rearrange("p (r f) -> p r f", r=2)[:, :, off:]
                nc.scalar.activation(p3, s3, mybir.ActivationFunctionType.Exp)
            else:
                nc.scalar.activation(pT[:], sps[:], mybir.ActivationFunctionType.Exp)
            pTm = None
            if 0 <= m:
                # chain-critical diag chunks of the last stretch mask on DVE
                # (shorter op, off the Pool q7-launch path); the rest on Pool
                eng = nc.vector if (qb == 3 and pair == 1) else nc.gpsimd
                pTm = pTmp.tile([128, 2 * KT], BF16, tag="pTm", name="pTm")
                for hh in range(2):
                    eng.tensor_mul(
                        pTm[:, hh * KT : (hh + 1) * KT],
                        pT[:, hh * SB + m * KT : hh * SB + (m + 1) * KT],
                        tri[:],
                    )
            return pT, pTm

        def emit_av(qb, pair, kt, oa, pT, pTm):
            """AV for chunk kt into oa; on the diagonal qsub, also emits
            normalization + transpose for that finished q-subtile."""
            m = kt - 4 * qb
            on_t = None
            if 0 <= m:
                on_t = onp.tile([128, KT], BF16, tag="on", name="on")
            for hh in range(2):
                h = 2 * pair + hh
                for qsub in range(4):
                    if qsub < m:
                        continue
                    if qsub == m:
                        lhs = pTm[:, hh * KT : (hh + 1) * KT]
                    else:
                        lhs = pT[:, hh * SB + qsub * KT : hh * SB + (qsub + 1) * KT]
                    # One accumulation group per PSUM bank (2KB zero region):
                    # start only on the first matmul into the bank, stop on
                    # the last. Per-region zero-init happens on first touch
                    # via the pending-zero mechanics.
                    nc.tensor.matmul(
                        oa[hh][:, qsub * VC : (qsub + 1) * VC],
                        lhs,
                        V[kt][:, h * VC : (h + 1) * VC],
                        start=(kt == 0 and qsub == 0),
                        stop=(kt == 4 * qb + 3 and qsub == 3),
                        skip_group_check=True,
                    )
            if 0 <= m:
                # q-subtile `m` just received its last contribution: normalize
                qsl_g = slice((qb * 4 + m) * KT, (qb * 4 + m + 1) * KT)
                for hh in range(2):
                    rc = rcp.tile([128, 1], F32, tag="rc", name="rc")
                    nc.vector.reciprocal(
                        rc[:], oa[hh][:, m * VC + HD : m * VC + VC]
                    )
                    nc.vector.tensor_scalar_mul(
                        on_t[:, hh * HD : (hh + 1) * HD],
                        oa[hh][:, m * VC : m * VC + HD],
                        rc[:],
                    )
                if qb == 3 and pair == 1:
                    # Final stretch: DMA-transpose latency (~2.4us) would
                    # gate the final out-projections; PE has slack here, so
                    # transpose on the PE via identity permutation. The
                    # out-projection is returned for deferred emission so
                    # the next chunk's scores aren't stuck behind it.
                    tp_ps = psW.tile([128, KT], BF16, tag="pw", name="tp_ps")
                    nc.tensor.matmul(
                        tp_ps[:], on_t[:], eye[:],
                        is_transpose=True, start=True, stop=True,
                        skip_group_check=True,
                    )
                    nc.vector.tensor_copy(OT[pair][:, qsl_g], tp_ps[:])
                    return lambda: oproj_final(12 + m)
                else:
                    nc.sync.dma_start_transpose(OT[pair][:, qsl_g], on_t[:])
            return None

        # ---------------- prologue: Q0/K0 (both t) chase x DMA arrivals ------
        # PE p-state ramps to full speed only after ~3us of CONTINUOUS busy;
        # any idle gap resets it. Keep PE spinning on throwaway matmuls while
        # input DMAs stream so the real work runs at 2.4GHz from the start.
        # Q0/K0 for BOTH pairs accumulate into the (idle) score PSUM slots:
        # 4 passes per d-tile arrival keeps PE saturated through the chase.
        # warm source comes from a memset (not a DMA) so PE can start
        # spinning at ~0.3us instead of waiting ~3us for the first DMA
        nc.vector.memset(wsrc[:], 1.0)

        def warm(n):
            for _ in range(n):
                wps = psW.tile([128, KT], F32, tag="pw", name="wps")
                nc.tensor.matmul(
                    wps[:], wsrc[:], wsrc[:], start=True, stop=True,
                    skip_group_check=True,
                )

        warm(44)
        psq = psS.tile([128, 2 * SB], F32, tag="sc", name="psq")
        psk = psS.tile([128, 2 * SB], F32, tag="sc", name="psk")
        for d in range(8):
            for t in range(2):
                nc.tensor.matmul(
                    psq[:, t * SB : (t + 1) * SB], w_tile(wq_sb, d, t),
                    x3[:, d, 0:SB],
                    start=(d == 0), stop=(d == 7), skip_group_check=True,
                )
                nc.tensor.matmul(
                    psk[:, t * SB : (t + 1) * SB], w_tile(wk_sb, d, t),
                    x3[:, d, 0:SB],
                    start=(d == 0), stop=(d == 7), skip_group_check=True,
                )
        # ACT is idle until the first exp — it takes the K copies so the
        # first score chunk isn't serialized behind a DVE copy queue.
        nc.scalar.copy(KTt[0][:, 0:SB], psk[:, 0:SB])
        nc.vector.tensor_scalar_mul(QT[0][:, 0:SB], psq[:, 0:SB], 0.125)
        nc.scalar.copy(KTt[1][:, 0:SB], psk[:, SB:])
        nc.vector.tensor_scalar_mul(QT[1][:, 0:SB], psq[:, SB:], 0.125)

        # ---------------- global schedule ----------------
        # Scores/exp for ALL chunks flow as one global stream (decoupled
        # from AV consumption via the SBUF pT ring) so ACT never starves at
        # (qb, pair) stretch boundaries. AV lags LAG chunks behind and
        # switches oa PSUM groups as its pointer crosses stretch borders.
        # Fillers are pinned to global chunk-slots, respecting emission-
        # order dependencies (V(st) before its first AV, Q/K(blk, t) before
        # that stretch's first scores, oproj after its OT transposes).
        def pv(st):
            return lambda: proj_v(st)

        def pq(blk, t):
            return lambda: proj_qk(wq_sb, QT, blk, t, 0.125)

        def pk(blk, t):
            return lambda: proj_qk(wk_sb, KTt, blk, t, None)

        def po(st):
            return lambda: oproj(st)

        unit_map = {}
        for st in range(16):
            unit_map[f"v{st}"] = pv(st)
            if st < 12:
                unit_map[f"o{st}"] = po(st)
        for blk in range(4):
            for t in range(2):
                unit_map[f"q{blk}{t}"] = pq(blk, t)
                unit_map[f"k{blk}{t}"] = pk(blk, t)
        fillers = [(s, unit_map[u]) for s, u in CONFIG["fillers"]]
        fillers.sort(key=lambda x: x[0])

        chunks = [
            (qb, pair, kt)
            for qb, pair in CONFIG["stretch_order"]
            for kt in range(4 * (qb + 1))
        ]
        LAG = CONFIG["lag"]
        filled = 0
        cur_group = None
        oa = None
        pending = deque()
        post_q = deque()

        def fill_to(s):
            nonlocal filled
            while filled < len(fillers) and fillers[filled][0] <= s:
                fillers[filled][1]()
                filled += 1

        def process_av():
            nonlocal cur_group, oa
            (qb_, pair_, kt_), pT_, pTm_ = pending.popleft()
            if cur_group != (qb_, pair_):
                cur_group = (qb_, pair_)
                oa = [
                    psO.tile(
                        [128, 4 * VC], F32, tag=f"oa{hh}", name=f"oa{hh}",
                        padded_shape=[128, SB],
                    )
                    for hh in range(2)
                ]
            post = emit_av(qb_, pair_, kt_, oa, pT_, pTm_)
            if post is not None:
                post_q.append(post)

        lag_tail = CONFIG.get("lag_tail", LAG)
        tail_from = CONFIG.get("tail_from", len(chunks) + 1)
        si = 0
        for si, (qb, pair, kt) in enumerate(chunks, start=1):
            pT, pTm = emit_chunk(qb, pair, kt)
            pending.append(((qb, pair, kt), pT, pTm))
            fill_to(si)
            while post_q:
                post_q.popleft()()
            while len(pending) > (lag_tail if si >= tail_from else LAG):
                process_av()
        while pending:
            process_av()
            si += 1
            fill_to(si)
            while post_q:
                post_q.popleft()()
        fill_to(10**9)

        # (st 12-15 out-projections are emitted inline at qb3/pair1 diag AVs)

    nc.compile()
    return nc


_NC = None


def _get_nc():
    global _NC
    if _NC is None:
        _NC = build_nc()
    return _NC


def make_in_maps(x, q_w, k_w, v_w, o_w):
    kk = np.arange(KT)[:, None]
    qq = np.arange(KT)[None, :]
    tri = (kk <= qq).astype(ml_dtypes.bfloat16)
    in_maps = []
    for c in range(N_CORES):
        b, g = c // 4, c % 4
        rows = slice(g * DQ, (g + 1) * DQ)
        woT = np.ascontiguousarray(o_w[:, g * DQ : (g + 1) * DQ].T).reshape(2, 128, D)
        eye = np.eye(KT, dtype=ml_dtypes.bfloat16)
        in_maps.append(
            {
                "xT": np.ascontiguousarray(x[b].T).astype(ml_dtypes.bfloat16),
                "wqT": np.ascontiguousarray(q_w[rows, :].T).astype(ml_dtypes.bfloat16),
                "wkT": np.ascontiguousarray(k_w[rows, :].T).astype(ml_dtypes.bfloat16),
                "wvT": np.ascontiguousarray(v_w[rows, :].T).astype(ml_dtypes.bfloat16),
                "woT": woT.astype(ml_dtypes.bfloat16),
                "tri": tri,
                "eye": eye,
            }
        )
    return in_maps


def run(x, q_w, k_w, v_w, o_w, trace=False, **spmd_kwargs):
    nc = _get_nc()
    in_maps = make_in_maps(
        np.asarray(x, dtype=np.float32),
        np.asarray(q_w, dtype=np.float32),
        np.asarray(k_w, dtype=np.float32),
        np.asarray(v_w, dtype=np.float32),
        np.asarray(o_w, dtype=np.float32),
    )
    res = run_bass_kernel_spmd(
        nc, in_maps, core_ids=list(range(N_CORES)), trace=trace, **spmd_kwargs
    )
    parts = [np.asarray(r["y"], dtype=np.float32) for r in res.results]
    out = np.empty((B, S, D), dtype=np.float32)
    for b in range(B):
        out[b] = parts[b * 4] + parts[b * 4 + 1] + parts[b * 4 + 2] + parts[b * 4 + 3]
    return out, res


def kernel(x, q_w, k_w, v_w, o_w):
    out, _ = run(x, q_w, k_w, v_w, o_w, trace=False)
    return out
